# revision 2
# baseline (speedup 1.0000x reference)
"""Trainium2 Bass kernel for nn_CausalSelfAttention_38216619000057.

Reference semantics (faithful to the source bug q = k):
    qkv = x @ W_attn + b_attn ; _, k, v = split(qkv)
    S = (K K^T) * D**-0.5  (per head, causal-masked), P = softmax(S)
    out = (P V) reshaped @ W_proj + b_proj

Sharding over 8 cores: data-parallel on B (4), tensor-parallel on heads (2
groups of 8). Core c handles batch c//2, heads 8*(c%2)..8*(c%2)+7, and
produces a partial projection output; the host sums the two partials per
batch and adds b_proj + b_v @ W_proj (the V-bias contribution commutes
through softmax because rows of P sum to 1).

Since q = k, S is symmetric, so S^T tiles (keys on partitions, queries on
the free axis) are computed directly from the same K^T operand, which is
what the P V matmul needs as its moving operand -- no on-chip transposes of
the attention matrix. The causal mask is applied on the exp(S^T) tiles with
a Pool-engine affine_select (fill 0) over the masked triangle of each
diagonal-crossing block; the softmax denominators come for free from a
column of ones appended to V (the ones-row of the PV accumulation sums
exactly the surviving E entries).

All matmul operands are float32r (TF32-like fast mode). Heads within a pair
are packed on partitions 0:64 / 64:128 of the same K^T tile and their S^T
matmuls are issued interleaved so the PE's row-group concurrency can
overlap the two 64-contraction matmuls.

Input DMAs are split across the two HWDGE queues (x on SP, weights on
Activation) so the first x tile lands immediately at kernel start.
"""

import threading

import numpy as np

import concourse.bacc as bacc
import concourse.mybir as mybir
import concourse.tile as tile
from concourse.bass_utils import run_bass_kernel_spmd
from concourse.masks import make_identity

B, T, D = 4, 2048, 1024
H = 16
HD = 64
NCORES = 8
HPC = 8  # heads per core
ISQ = float(D**-0.5) ** 0.5  # K is pre-scaled by sqrt(D**-0.5)
F32 = mybir.dt.float32
F32R = mybir.dt.float32r
BF16 = mybir.dt.bfloat16

Ident = mybir.ActivationFunctionType.Identity
Exp = mybir.ActivationFunctionType.Exp
Mult = mybir.AluOpType.mult
IsGe = mybir.AluOpType.is_ge

_cache_lock = threading.Lock()
_cached_nc = {}


def _declare_io(nc, synth=False):
    kind = "Internal" if synth else "ExternalInput"
    ts = {}
    ts["x"] = nc.dram_tensor("x", [T, D], F32, kind=kind)
    ts["wk"] = nc.dram_tensor("wk", [128, 4, 8, 128], F32R, kind=kind)
    ts["wv"] = nc.dram_tensor("wv", [128, 8, 512], F32R, kind=kind)
    ts["wp"] = nc.dram_tensor("wp", [128, 4, 1024], F32R, kind=kind)
    ts["bk"] = nc.dram_tensor("bk", [128, 4], F32, kind=kind)
    ts["out"] = nc.dram_tensor("out", [T, D], F32, kind="Internal" if synth else "ExternalOutput")
    if synth:
        ts["done"] = nc.dram_tensor("done", [1, 4], F32, kind="ExternalOutput")
    return ts


def _synth_init(nc, tc, io):
    """Fill the Internal input tensors with benign constants on device."""
    with tc.tile_pool(name="init", bufs=1) as pool:
        it = pool.tile([128, 4096], F32, name="init_t")
        nc.vector.memset(it[:], 0.01)
        for tb in range(16):
            nc.sync.dma_start(io["x"][tb * 128 : (tb + 1) * 128, :], it[:, 0:1024])
        nc.sync.dma_start(io["wk"][:], it[:, 0 : 4 * 8 * 128].bitcast(F32R).rearrange("p (a b c) -> p a b c", a=4, b=8))
        nc.sync.dma_start(io["wv"][:], it[:, 0 : 8 * 512].bitcast(F32R).rearrange("p (a b) -> p a b", a=8))
        nc.sync.dma_start(io["wp"][:], it[:, 0 : 4 * 1024].bitcast(F32R).rearrange("p (a b) -> p a b", a=4))
        nc.sync.dma_start(io["bk"][:], it[:, 0:4])


def _emit_body(nc, tc, io, g):
    """One full forward pass. g holds the persistent SBUF tiles."""
    kt_sb, v_ones = g["kt_sb"], g["v_ones"]
    wp_sb, bk_sb = g["wp_sb"], g["bk_sb"]
    ident, ones_sb = g["ident"], g["ones_sb"]
    x, out = io["x"], io["out"]

    # ---------- Phase A: x^T (PE transpose), K^T, V ----------
    with (
        tc.tile_pool(name="wkv", bufs=1) as wkv,
        tc.tile_pool(name="xload", bufs=4) as xload,
        tc.tile_pool(name="xtp", bufs=2) as xtp,
        tc.tile_pool(name="psT", bufs=3, space="PSUM") as psT,
        tc.tile_pool(name="psKV", bufs=4, space="PSUM") as psKV,
    ):
        wk_sb = wkv.tile([128, 4, 8, 128], F32R)
        wv_sb = wkv.tile([128, 8, 512], F32R)
        # weights go on the Activation HWDGE queue; x tiles on the SP queue,
        # so the first transposes aren't stuck behind 2.5MB of weights.
        nc.scalar.dma_start(wk_sb[:], io["wk"][:])
        nc.scalar.dma_start(wv_sb[:], io["wv"][:])

        for tci in range(4):  # t-chunks of 512
            xt_chunk = xtp.tile([128, 8, 512], F32R, tag="xtc")
            for tbl in range(4):
                xt_ = xload.tile([128, D], F32, tag="xl")
                tb = 4 * tci + tbl
                nc.sync.dma_start(xt_[:], x[tb * 128 : (tb + 1) * 128, :])
                for gg in (0, 1):  # groups of 4 e-blocks
                    tps = psT.tile([128, 512], F32, tag="tp")
                    for ebl in range(4):
                        eb = 4 * gg + ebl
                        nc.tensor.transpose(
                            tps[:, ebl * 128 : (ebl + 1) * 128],
                            xt_[:, eb * 128 : (eb + 1) * 128],
                            ident[:],
                        )
                    nc.vector.tensor_copy(
                        xt_chunk[:, 4 * gg : 4 * gg + 4, tbl * 128 : (tbl + 1) * 128],
                        tps[:].rearrange("p (e c) -> p e c", c=128),
                    )
            # K^T for this t-chunk
            for hp in range(4):
                kps = psKV.tile([128, 512], F32, tag="kv")
                for eb in range(8):
                    nc.tensor.matmul(
                        kps[:],
                        wk_sb[:, hp, eb, :],
                        xt_chunk[:, eb, :],
                        start=(eb == 0),
                        stop=(eb == 7),
                    )
                nc.scalar.activation(
                    kt_sb[:, hp, tci * 512 : (tci + 1) * 512],
                    kps[:],
                    Ident,
                    bias=bk_sb[:, hp : hp + 1],
                    scale=ISQ,
                )
            # V rows for this t-chunk
            for tbl in range(4):
                vps = psKV.tile([128, 512], F32, tag="kv")
                for eb in range(8):
                    nc.tensor.matmul(
                        vps[:],
                        xt_chunk[:, eb, tbl * 128 : (tbl + 1) * 128],
                        wv_sb[:, eb, :],
                        start=(eb == 0),
                        stop=(eb == 7),
                    )
                tb = 4 * tci + tbl
                nc.vector.tensor_copy(
                    v_ones[:, tb, :].rearrange("p (h c) -> p h c", c=65)[:, :, 0:64],
                    vps[:].rearrange("p (h c) -> p h c", c=64),
                )

    # ---------- Phase B: attention, with per-chunk fused projection ----------
    with tc.tile_pool(name="obig", bufs=1) as obig:
        o_t = obig.tile([128, 4, T], F32R, name="o_t")
        with (
            tc.tile_pool(name="ps_s", bufs=3, space="PSUM") as ps_s,
            tc.tile_pool(name="ps_pv", bufs=2, space="PSUM") as ps_pv,
            tc.tile_pool(name="ebuf", bufs=6) as ebuf,
            tc.tile_pool(name="rbuf", bufs=6) as rbuf,
            tc.tile_pool(name="obuf", bufs=3) as obuf,
        ):
            for ci in range(4):
                njb = 4 * ci + 4
                for hp in range(4):
                    rhs = [
                        kt_sb[64 * q : 64 * q + 64, hp, ci * 512 : (ci + 1) * 512]
                        for q in (0, 1)
                    ]
                    pv = [
                        ps_pv.tile([65, 512], F32, tag="pv", name="pv") for _ in (0, 1)
                    ]
                    for jbp in range(njb // 2):
                        sps = [
                            ps_s.tile([128, 1024], F32, tag="s", name="sps")
                            for _ in (0, 1)
                        ]
                        for half in (0, 1):
                            jb = 2 * jbp + half
                            hs = slice(half * 512, half * 512 + 512)
                            for q in (0, 1):  # adjacent MMs hit distinct row groups
                                nc.tensor.matmul(
                                    sps[q][:, hs],
                                    kt_sb[
                                        64 * q : 64 * q + 64,
                                        hp,
                                        jb * 128 : (jb + 1) * 128,
                                    ],
                                    rhs[q],
                                    start=True,
                                    stop=True,
                                )
                        eps = []
                        for q in (0, 1):
                            ep = ebuf.tile([128, 1024], F32R, tag="e")
                            nc.scalar.activation(ep[:], sps[q][:], Exp)
                            eps.append(ep)
                        for half in (0, 1):
                            jb = 2 * jbp + half
                            if jb >= 4 * ci:
                                # diagonal-crossing block: zero the strictly-
                                # upper (j > i) triangle of exp(S^T) on Pool.
                                # Masked region is cols [0, 128*(oi+1)) only.
                                oi = jb - 4 * ci
                                ncols = 128 * (oi + 1)
                                for q in (0, 1):
                                    nc.gpsimd.affine_select(
                                        out=eps[q][:, half * 512 : half * 512 + ncols],
                                        in_=eps[q][:, half * 512 : half * 512 + ncols],
                                        pattern=[[1, ncols]],
                                        compare_op=IsGe,
                                        fill=0.0,
                                        base=-128 * oi,
                                        channel_multiplier=-1,
                                    )
                        for half in (0, 1):
                            jb = 2 * jbp + half
                            hs = slice(half * 512, half * 512 + 512)
                            for q in (0, 1):
                                hl = 2 * hp + q
                                nc.tensor.matmul(
                                    pv[q][:],
                                    v_ones[:, jb, 65 * hl : 65 * hl + 65],
                                    eps[q][:, hs],
                                    start=(jb == 0),
                                    stop=(jb == njb - 1),
                                )
                    for q in (0, 1):
                        r_row = rbuf.tile([1, 512], F32R, tag="rr")
                        with nc.allow_low_precision(
                            reason="f32r reciprocal output feeds bc matmul"
                        ):
                            nc.vector.reciprocal(r_row[:], pv[q][64:65, :])
                        bcps = ps_s.tile([64, 512], F32, tag="s", name="bcps")
                        nc.tensor.matmul(
                            bcps[:], ones_sb[:], r_row[:], start=True, stop=True
                        )
                        r_bc = rbuf.tile([64, 512], F32, tag="rb")
                        nc.vector.tensor_copy(r_bc[:], bcps[:])
                        nc.vector.tensor_tensor(
                            o_t[64 * q : 64 * q + 64, hp, ci * 512 : (ci + 1) * 512],
                            pv[q][0:64, :],
                            r_bc[:],
                            Mult,
                        )
                # fused output projection for this chunk's 4 t-blocks
                for tbl in range(4):
                    tb = 4 * ci + tbl
                    for nch in range(2):
                        ops_ = ps_s.tile([128, 512], F32, tag="s", name="ops")
                        for hp2 in range(4):
                            nc.tensor.matmul(
                                ops_[:],
                                o_t[:, hp2, tb * 128 : (tb + 1) * 128],
                                wp_sb[:, hp2, nch * 512 : (nch + 1) * 512],
                                start=(hp2 == 0),
                                stop=(hp2 == 3),
                            )
                        ob = obuf.tile([128, 512], F32, tag="ob")
                        nc.vector.tensor_copy(ob[:], ops_[:])
                        nc.sync.dma_start(
                            out[
                                tb * 128 : (tb + 1) * 128, nch * 512 : (nch + 1) * 512
                            ],
                            ob[:],
                        )

def _build_program(nreps: int = 1, synth: bool = False):
    nc = bacc.Bacc("TRN2", target_bir_lowering=False)
    io = _declare_io(nc, synth=synth)

    with tile.TileContext(nc) as tc:
        if synth:
            _synth_init(nc, tc, io)
        with tc.tile_pool(name="singles", bufs=1) as singles:
            g = {}
            g["kt_sb"] = singles.tile([128, 4, T], F32R, name="kt_sb")
            g["v_ones"] = singles.tile([128, 16, HPC * 65], F32R, name="v_ones")
            g["wp_sb"] = singles.tile([128, 4, 1024], F32R, name="wp_sb")
            g["bk_sb"] = singles.tile([128, 4], F32, name="bk_sb")
            g["ident"] = singles.tile([128, 128], F32, name="ident")
            g["ones_sb"] = singles.tile([1, 64], F32R, name="ones_sb")

            # wp is only needed in phase B -- keep it off both HWDGE queues
            # (SWDGE via Pool) so it can't delay x or wk/wv at startup.
            nc.gpsimd.dma_start(g["wp_sb"][:], io["wp"][:])
            nc.scalar.dma_start(g["bk_sb"][:], io["bk"][:])
            make_identity(nc, g["ident"][:])
            nc.vector.memset(g["ones_sb"][:].bitcast(F32), 1.0)
            nc.vector.memset(
                g["v_ones"][:]
                .bitcast(F32)
                .rearrange("p t (h c) -> p t h c", c=65)[:, :, :, 64:65],
                1.0,
            )

            for _rep in range(nreps):
                _emit_body(nc, tc, io, g)

            if synth:
                with tc.tile_pool(name="fin", bufs=1) as fin:
                    dn = fin.tile([1, 4], F32, name="dn")
                    nc.vector.memset(dn[:], 1.0)
                    nc.sync.dma_start(io["done"][:], dn[:])

    nc.compile()
    return nc


def _build_null_program():
    """Same I/O signature, trivial body -- for wall-clock differencing."""
    nc = bacc.Bacc("TRN2", target_bir_lowering=False)
    io = _declare_io(nc)
    with tile.TileContext(nc) as tc:
        with tc.tile_pool(name="sb", bufs=2) as sb:
            t = sb.tile([128, 512], F32)
            nc.sync.dma_start(t[:], io["x"][0:128, 0:512])
            for tb in range(16):
                for nch in range(2):
                    nc.sync.dma_start(
                        io["out"][
                            tb * 128 : (tb + 1) * 128, nch * 512 : (nch + 1) * 512
                        ],
                        t[:],
                    )
    nc.compile()
    return nc


def _get_program(nreps: int = 1, synth: bool = False):
    with _cache_lock:
        key = (nreps, synth)
        if key not in _cached_nc:
            _cached_nc[key] = _build_program(nreps, synth)
        return _cached_nc[key]


def _core_inputs(c, x, W_attn, b_attn):
    b = c // 2
    h0 = HPC * (c % 2)
    c0k = D + h0 * HD
    c0v = 2 * D + h0 * HD
    wk_np = np.ascontiguousarray(
        W_attn[:, c0k : c0k + 512].reshape(8, 128, 4, 128).transpose(1, 2, 0, 3)
    )
    wv_np = np.ascontiguousarray(
        W_attn[:, c0v : c0v + 512].reshape(8, 128, 512).transpose(1, 0, 2)
    )
    bk_np = np.ascontiguousarray(b_attn[c0k : c0k + 512].reshape(4, 128).T * ISQ)
    return {
        "x": np.ascontiguousarray(x[b]),
        "wk": wk_np,
        "wv": wv_np,
        "bk": bk_np,
    }


def _core_wp(c, W_proj):
    h0 = HPC * (c % 2)
    r0 = h0 * HD
    return np.ascontiguousarray(
        W_proj[r0 : r0 + 512, :].reshape(4, 128, 1024).transpose(1, 0, 2)
    )


def kernel(x, W_attn, b_attn, W_proj, b_proj, **_unused):
    x = np.asarray(x, dtype=np.float32)
    W_attn = np.asarray(W_attn, dtype=np.float32)
    b_attn = np.asarray(b_attn, dtype=np.float32)
    W_proj = np.asarray(W_proj, dtype=np.float32)
    b_proj = np.asarray(b_proj, dtype=np.float32)

    nc = _get_program()
    in_maps = []
    for c in range(NCORES):
        m = _core_inputs(c, x, W_attn, b_attn)
        m["wp"] = _core_wp(c, W_proj)
        in_maps.append(m)

    res = run_bass_kernel_spmd(nc, in_maps, core_ids=list(range(NCORES)))

    bias_row = b_proj + b_attn[2 * D : 3 * D] @ W_proj
    out = np.empty((B, T, D), dtype=np.float32)
    for b in range(B):
        out[b] = res.results[2 * b]["out"] + res.results[2 * b + 1]["out"] + bias_row
    return out
